# revision 63
# baseline (speedup 1.0000x reference)
"""MoE expert-gating kernel for 8 Trainium2 NeuronCores.

Problem (nn_ExpertGating): router MLP (H->H relu, H->E) + softmax + top-2
gating + weighted combine of per-expert outputs.

Sharding: data-parallel over the B*S=8192 tokens -> 1024 tokens per core.
Each core runs the full router for its tokens and combines its slice of all
8 experts' outputs.  No collectives needed; host concatenates the slices.

v3 structure (baseline was 152us, v2 140us):
  * Inputs arrive as fused u16 blobs (fp16 hi | bf16 lo interleaved per
    k-slice) bitcast to f16/bf16 SBUF views -- halves dma_start count
    (~610ns dispatch each on the issuing engine queue).
  * Criticality-ordered, ring-balanced delivery: sync ring carries seg0's
    four x k-pair tiles then xs1/xs3/xs4; scalar ring carries W1 (k0, k1,
    then pairs), the constants blob, then xs2.  First matmul needs only
    x-pair0 + W1-k0 (~0.8MB), so the PE starts right after the ~7us
    framework preamble + DMA latency instead of waiting for 9MB.
  * seg0 runs k-outer over two m-halves (PSUM accumulators per half) so
    it can consume x/W1 k-slices as they land; later segments m-outer.
  * Stage-3 (W2, fp32) matmuls are batched per segment into one burst in
    the next segment's m=0 slot: entering/leaving fp32 matmul mode costs
    ~0.6us of PE pipeline each time (measured 212+432ns), so per-m
    interleaving pays it 8x per segment, the burst once.
  * Segments are (2,2,2,1,1) chunks wide: the last two are single-chunk
    so the final token chunk's gather+combine tail is half as deep, and
    the second-to-last chunk's tail overlaps the last segment's compute.
  * Combines pop one per slot while >=3 chunks are pending; output is
    written f16 (host upcasts) to halve the output DMA.

fp16x3 passes (hi*hi, lo_w*hi_x, hi_w*lo_x) are required: the min top-2/3
prob margin on this data is ~5e-6; fewer passes (or fp8 cross terms)
misrank tokens, and a single misranked token blows the absmax budget.
"""

import numpy as np

B, S, H, E = 4, 2048, 1024, 8
N_CORES = 8
T = (B * S) // N_CORES  # tokens per core
P = 128  # partitions
TCH = T // P  # token chunks per core (8)
KT = H // P  # contraction tiles (8)
HAL = 512  # psum pad width (full bank)
SEGS = [(0, 4), (4, 6), (6, 7), (7, 8)]
NSEG = len(SEGS)
SW = 4 * P  # seg0 width (tokens); N=512 matmuls are the PE's best ratio
# b1 | w2 | ident | b2 | per-chunk gather row-base bits = 89 cols
CBLOB = KT + KT * E + E + 1 + TCH

_compiled_nc = None


def _build():
    import concourse.bacc as bacc
    import concourse.bass as bass
    import concourse.tile as tile
    from concourse import mybir

    f32 = mybir.dt.float32
    f16 = mybir.dt.float16
    bf16 = mybir.dt.bfloat16
    u16 = mybir.dt.uint16
    u32 = mybir.dt.uint32
    nc = bacc.Bacc("TRN2", target_bir_lowering=False, debug=False,
                   num_devices=N_CORES)

    segw = [(c1 - c0) * P for c0, c1 in SEGS]
    # seg0 x: per-k tiles [p, hl, u] u16
    xk = nc.dram_tensor("xk", [KT, P, 2, SW], u16, kind="ExternalInput").ap()
    xsd = [nc.dram_tensor(f"xs{s}", [P, KT, 2, segw[s]], u16,
                          kind="ExternalInput").ap()
           for s in range(1, NSEG)]
    # w1: per-k tiles [p, hl, m] u16 (k0 delivered as hi/lo halves)
    w1r = nc.dram_tensor("w1r", [KT, P, 2, H], u16, kind="ExternalInput").ap()
    eo = nc.dram_tensor("eo", [E * T, H], f16, kind="ExternalInput").ap()
    cblob = nc.dram_tensor("cblob", [P, CBLOB], f32, kind="ExternalInput").ap()
    out = nc.dram_tensor("out", [T, H], f16, kind="ExternalOutput").ap()

    with tile.TileContext(nc) as tc:
        with (
            tc.tile_pool(name="singles", bufs=1) as singles,
            tc.tile_pool(name="eopool", bufs=6) as eopool,
            tc.tile_pool(name="accpool", bufs=4) as accpool,
            tc.tile_pool(name="smalls", bufs=8) as smalls,
            tc.tile_pool(name="ltpool", bufs=2) as ltpool,
            tc.tile_pool(name="psum", bufs=8, space="PSUM") as psum,
        ):
            # ---- input DMAs: per-k granularity, need-ordered and
            # byte-balanced across both HWDGE rings, so seg0's k-outer loop
            # never waits long on any single transfer ----
            xk_t = {}
            w1_t = {}   # k -> tile [P, 2, H] (k<3: separate [P, H] halves)
            w1_s = {}   # (k, hl) -> tile for split k's
            xk_s = {}

            def w1_split_dma(ring, k, hl):
                t = singles.tile([P, H], u16, tag="w1s", name=f"w1s{k}_{hl}",
                                 bufs=6)
                ring.dma_start(out=t[:], in_=w1r[k, :, hl, :])
                w1_s[(k, hl)] = t

            def xk_split_dma(ring, k, hl):
                t = singles.tile([P, SW], u16, tag="xks", name=f"xks{k}_{hl}",
                                 bufs=6)
                ring.dma_start(out=t[:], in_=xk[k, :, hl, :])
                xk_s[(k, hl)] = t

            def xk_dma(ring, k):
                t = singles.tile([P, 2, SW], u16, tag="xk", name=f"xk{k}",
                                 bufs=KT)
                ring.dma_start(out=t[:], in_=xk[k])
                xk_t[k] = t

            def w1_dma(ring, k):
                t = singles.tile([P, 2, H], u16, tag="w1", name=f"w1_{k}",
                                 bufs=KT - 1)
                ring.dma_start(out=t[:], in_=w1r[k])
                w1_t[k] = t

            # need order: (x_k, w1_k) pairs; even w1 on scalar, odd on
            # sync; k0-k2 split hi|lo on both x and w1 so the k-outer loop
            # gates on 256KB hi halves instead of 512KB fused tiles
            xk_split_dma(nc.sync, 0, 0)
            xk_split_dma(nc.sync, 0, 1)
            w1_split_dma(nc.scalar, 0, 0)
            w1_split_dma(nc.scalar, 0, 1)
            w1_split_dma(nc.sync, 1, 0)
            w1_split_dma(nc.sync, 1, 1)
            xk_split_dma(nc.scalar, 1, 0)
            xk_split_dma(nc.scalar, 1, 1)
            xk_split_dma(nc.sync, 2, 0)
            xk_split_dma(nc.sync, 2, 1)
            w1_split_dma(nc.scalar, 2, 0)
            w1_split_dma(nc.scalar, 2, 1)
            w1_dma(nc.sync, 3)
            xk_dma(nc.scalar, 3)
            xk_dma(nc.sync, 4)
            w1_dma(nc.scalar, 4)
            w1_dma(nc.sync, 5)
            xk_dma(nc.scalar, 5)
            xk_dma(nc.sync, 6)
            w1_dma(nc.scalar, 6)
            w1_dma(nc.sync, 7)
            xk_dma(nc.scalar, 7)
            cb = singles.tile([P, CBLOB], f32)
            nc.scalar.dma_start(out=cb[:], in_=cblob)
            xs_t = {}
            for s, ring in ((1, nc.sync), (2, nc.scalar), (3, nc.sync)):
                t = singles.tile([P, KT, 2, segw[s]], u16, tag=f"xs{s}",
                                 name=f"xs{s}")
                ring.dma_start(out=t[:], in_=xsd[s - 1])
                xs_t[s] = t

            b1_sb = cb[:, 0:KT]                       # b1_sb[p,m] = b1[m*128+p]
            w2col = lambda j: cb[:, KT + j * E:KT + (j + 1) * E]  # W2[j*128+p, e]
            ident = cb[0:E, KT + KT * E:KT + KT * E + E]          # eye(8)
            b2_sb = cb[0:E, KT + KT * E + E:KT + KT * E + E + 1]
            BAS = KT + KT * E + E + 1
            rowbase = lambda tch: cb[:, BAS + tch:BAS + tch + 1].bitcast(u32)

            def xsl(si, k, hl):
                dt = f16 if hl == 0 else bf16
                if si == 0:
                    if k < 3:
                        return xk_s[(k, hl)][:, :].bitcast(dt)
                    return xk_t[k][:, hl, :].bitcast(dt)
                return xs_t[si][:, k, hl, :].bitcast(dt)

            def wsl(k, hl, msl):
                dt = f16 if hl == 0 else bf16
                if k < 3:
                    return w1_s[(k, hl)][:, msl].bitcast(dt)
                return w1_t[k][:, hl, msl].bitcast(dt)

            # NOTE: keeping h in fp32 and stage-3 in fp32 matmul mode is
            # deliberate: an fp16-split variant (h as fp16 hi+lo computed
            # with DVE residual ops) added ~30us of DVE activity, which
            # pushed the chip into the P0 power state and downclocked the
            # PE 2.4 -> 2.0 GHz -- a 20% slowdown on everything.
            hT = singles.tile([P, KT, T], f32)

            def mm3(ps, si, m, k, start, stop):
                msl = slice(m * P, (m + 1) * P)
                wh, wl = wsl(k, 0, msl), wsl(k, 1, msl)
                nc.tensor.matmul(ps[:], lhsT=wh, rhs=xsl(si, k, 0),
                                 start=start, stop=False)
                nc.tensor.matmul(ps[:], lhsT=wl, rhs=xsl(si, k, 0),
                                 start=False, stop=False)
                nc.tensor.matmul(ps[:], lhsT=wh, rhs=xsl(si, k, 1),
                                 start=False, stop=stop)

            # ---- deferred work queues ----
            # s3q: stage-3 (W2) matmuls for the finished segment, burst out
            # in the next segment's m=0 slot (one fp32 mode transition).
            # paq: per-chunk transpose+softmax+top2+gather-launch work.
            # pending: chunks whose gather is in flight, awaiting combine.
            s3q = []
            paq = []
            pending = []

            def emit_relu(ps, ps3, si, m):
                sl = slice(SEGS[si][0] * P, SEGS[si][1] * P)
                nc.scalar.activation(
                    out=hT[:, m, sl], in_=ps[:],
                    func=mybir.ActivationFunctionType.Relu,
                    bias=b1_sb[:, m:m + 1], scale=1.0)
                s3q.append((ps3, m, sl, si))

            def emit_stage3(ent):
                ps3, m, sl, si = ent
                nc.tensor.matmul(ps3[:], lhsT=w2col(m), rhs=hT[:, m, sl],
                                 start=(m == 0), stop=(m == KT - 1))
                if m == KT - 1:
                    c0, c1 = SEGS[si]
                    lT = ltpool.tile([E, segw[si]], f32, tag="lT", name="lT",
                                     padded_shape=[E, HAL])
                    nc.scalar.activation(
                        out=lT[:], in_=ps3[:],
                        func=mybir.ActivationFunctionType.Identity,
                        bias=b2_sb, scale=1.0)
                    for tch in range(c0, c1):
                        paq.append((lT, c0, tch))

            # phase A for one 128-token chunk: transpose logits, top-2 in
            # LOGIT space (softmax is monotone), launch the gathers, and
            # only then compute the softmax gates off the critical path.
            def chunk_phase_a(lT, c0, tch):
                a = tch - c0
                pl = psum.tile([P, E], f32, tag="pl", name="pl", bufs=1,
                               padded_shape=[P, HAL])
                nc.tensor.transpose(pl[:], lT[:, a * P:(a + 1) * P], ident)
                mx8 = smalls.tile([P, 8], f32, tag="mx8", name="mx8")
                nc.vector.max(mx8[:], pl[:])
                idx8 = smalls.tile([P, 8], u32, tag="idx8", name="idx8")
                nc.vector.max_index(idx8[:], mx8[:], pl[:])
                # flat eo row = expert*T + (tch*128 + partition)
                rows = smalls.tile([P, 2], u32, tag="rows", name="rows")
                for s in range(2):
                    nc.vector.scalar_tensor_tensor(
                        out=rows[:, s:s + 1], in0=idx8[:, s:s + 1],
                        scalar=float(T), in1=rowbase(tch),
                        op0=mybir.AluOpType.mult, op1=mybir.AluOpType.add)
                eo_g = eopool.tile([P, 2, H], f16, tag="eog", name="eog")
                # NOTE: a single [P,2]-offset indirect DMA compiles and
                # simulates but dies at runtime (NRT INTERNAL) -- keep two
                # single-offset gathers
                for s in range(2):
                    nc.gpsimd.indirect_dma_start(
                        out=eo_g[:, s, :], out_offset=None, in_=eo,
                        in_offset=bass.IndirectOffsetOnAxis(
                            ap=rows[:, s:s + 1], axis=0))
                # gates: g0 = 1/sum(exp(l - l_max)), g1 = exp(l2 - l_max)/sum
                negmax = smalls.tile([P, 1], f32, tag="negmax", name="negmax")
                nc.vector.tensor_scalar_mul(negmax[:], mx8[:, 0:1], -1.0)
                exps = smalls.tile([P, E], f32, tag="exps", name="exps")
                nc.scalar.activation(exps[:], pl[:],
                                     func=mybir.ActivationFunctionType.Exp,
                                     bias=negmax[:], scale=1.0)
                ssum = smalls.tile([P, 1], f32, tag="ssum", name="ssum")
                nc.vector.reduce_sum(ssum[:], exps[:], axis=mybir.AxisListType.X)
                g0 = smalls.tile([P, 1], f32, tag="g0", name="g0")
                nc.vector.reciprocal(g0[:], ssum[:])
                g1e = smalls.tile([P, 1], f32, tag="g1e", name="g1e")
                nc.scalar.activation(g1e[:], mx8[:, 1:2],
                                     func=mybir.ActivationFunctionType.Exp,
                                     bias=negmax[:], scale=1.0)
                g1 = smalls.tile([P, 1], f32, tag="g1", name="g1")
                nc.vector.tensor_mul(g1[:], g1e[:], g0[:])
                pending.append((tch, eo_g, g0, g1))

            # phase B: weighted combine + f16 output store
            def chunk_phase_b(st, flush=False):
                tch, eo_g, g0, g1 = st
                acc = accpool.tile([P, H], f16, tag="acc", name="acc")
                osl = slice(tch * P, (tch + 1) * P)
                if flush:
                    # tail: h-halves in parallel (scalar ACT || DVE mul),
                    # each half's output DMA as soon as it lands
                    h0, h1 = slice(0, H // 2), slice(H // 2, H)
                    nc.scalar.activation(acc[:, h0], eo_g[:, 0, h0],
                                         func=mybir.ActivationFunctionType.Copy,
                                         scale=g0[:])
                    nc.vector.tensor_scalar_mul(acc[:, h1], eo_g[:, 0, h1],
                                                g0[:])
                    for half, ring in ((h0, nc.sync), (h1, nc.scalar)):
                        nc.vector.scalar_tensor_tensor(
                            out=acc[:, half], in0=eo_g[:, 1, half],
                            scalar=g1[:], in1=acc[:, half],
                            op0=mybir.AluOpType.mult, op1=mybir.AluOpType.add)
                        ring.dma_start(out=out[osl, half], in_=acc[:, half])
                else:
                    nc.vector.tensor_scalar_mul(acc[:], eo_g[:, 0, :], g0[:])
                    nc.vector.scalar_tensor_tensor(
                        out=acc[:], in0=eo_g[:, 1, :], scalar=g1[:], in1=acc[:],
                        op0=mybir.AluOpType.mult, op1=mybir.AluOpType.add)
                    nc.sync.dma_start(out=out[osl, :], in_=acc[:])

            def slot(m):
                # one non-burst slot after a stage-2 block
                if m == 0:
                    while s3q:  # previous segment's stage3s, one fp32 burst
                        emit_stage3(s3q.pop(0))
                else:
                    if paq:
                        lT, c0, tch = paq.pop(0)
                        chunk_phase_a(lT, c0, tch)
                    if len(pending) >= 3:
                        chunk_phase_b(pending.pop(0))

            # ---- segment 0: k-outer over two m-halves, pass-major inside
            # each k (so the hi-pass matmuls can start before that k's lo
            # halves land; x and W1 k-slices are consumed as delivered) ----
            ps3_seg0 = None
            for half in range(2):
                ms = range(half * 4, half * 4 + 4)
                ps_m = {m: psum.tile([P, SW], f32, tag="ps2", name=f"ps{m}",
                                     bufs=5, padded_shape=[P, HAL])
                        for m in ms}
                for k in range(KT):
                    for pss in range(3):
                        xop = xsl(0, k, 0 if pss < 2 else 1)
                        for m in ms:
                            wop = wsl(k, 1 if pss == 1 else 0,
                                      slice(m * P, (m + 1) * P))
                            nc.tensor.matmul(
                                ps_m[m][:], lhsT=wop, rhs=xop,
                                start=(k == 0 and pss == 0),
                                stop=(k == KT - 1 and pss == 2))
                if half == 0:
                    ps3_seg0 = psum.tile([E, SW], f32, tag="ps3", name="ps3",
                                         bufs=2, padded_shape=[E, HAL])
                for m in ms:
                    emit_relu(ps_m[m], ps3_seg0, 0, m)

            # ---- segments 1+: m-outer, pipelined slots ----
            for si in range(1, NSEG):
                ps3 = psum.tile([E, segw[si]], f32, tag="ps3", name="ps3",
                                bufs=2, padded_shape=[E, HAL])
                for m in range(KT):
                    ps = psum.tile([P, segw[si]], f32, tag="ps2", name="ps",
                                   bufs=5, padded_shape=[P, HAL])
                    for k in range(KT):
                        mm3(ps, si, m, k, k == 0, k == KT - 1)
                    slot(m)
                    emit_relu(ps, ps3, si, m)

            # ---- tail: last segment's stage3 burst, phase_a, flush ----
            while s3q:
                emit_stage3(s3q.pop(0))
            while paq:
                lT, c0, tch = paq.pop(0)
                chunk_phase_a(lT, c0, tch)
            while pending:
                chunk_phase_b(pending.pop(0), flush=True)

    nc.compile()
    return nc


def _get_nc():
    global _compiled_nc
    if _compiled_nc is None:
        _compiled_nc = _build()
    return _compiled_nc


def _split_hi_lo(a):
    """fp16 hi + bf16 lo split of an fp32 array (lo unscaled; bf16's
    exponent range covers it)."""
    import ml_dtypes
    a = np.asarray(a, dtype=np.float32)
    hi = a.astype(np.float16)
    lo = (a.astype(np.float64) - hi.astype(np.float64)).astype(ml_dtypes.bfloat16)
    return hi, lo


def make_in_maps(hidden_states, expert_outputs, W1, b1, W2, b2):
    hs = np.ascontiguousarray(np.asarray(hidden_states, dtype=np.float32)).reshape(B * S, H)
    eo = np.asarray(expert_outputs, dtype=np.float32).reshape(E, B * S, H)
    w1hi, w1lo = _split_hi_lo(W1)
    # w1 blob [k, p, hl, m] u16 = bits of W1[(k*128+p), m] hi/lo
    w1u = np.empty((KT, P, 2, H), dtype=np.uint16)
    w1u[:, :, 0, :] = w1hi.reshape(KT, P, H).view(np.uint16)
    w1u[:, :, 1, :] = w1lo.reshape(KT, P, H).view(np.uint16)
    b1v = np.asarray(b1, dtype=np.float32)
    w2 = np.asarray(W2, dtype=np.float32)
    b2v = np.asarray(b2, dtype=np.float32)
    # constants blob: b1 | w2 | ident | b2 | per-chunk gather row-base bits
    cblk = np.zeros((P, CBLOB), dtype=np.float32)
    cblk[:, 0:KT] = b1v.reshape(KT, P).T                    # b1[m*128+p]
    cblk[:, KT:KT + KT * E] = w2.reshape(KT, P, E).transpose(1, 0, 2).reshape(P, KT * E)
    cblk[0:E, KT + KT * E:KT + KT * E + E] = np.eye(E, dtype=np.float32)
    cblk[0:E, KT + KT * E + E] = b2v
    bas = KT + KT * E + E + 1
    for tch in range(TCH):
        cblk[:, bas + tch] = (np.arange(P, dtype=np.uint32)
                              + np.uint32(tch * P)).view(np.float32)
    in_maps = []
    for c in range(N_CORES):
        sl = slice(c * T, (c + 1) * T)
        xhi, xlo = _split_hi_lo(hs[sl].T)  # [H, T]

        def blob(c0, c1):
            w = (c1 - c0) * P
            u = np.empty((P, KT, 2, w), dtype=np.uint16)
            u[:, :, 0, :] = (xhi[:, c0 * P:c1 * P].reshape(KT, P, w)
                             .transpose(1, 0, 2).view(np.uint16))
            u[:, :, 1, :] = (xlo[:, c0 * P:c1 * P].reshape(KT, P, w)
                             .transpose(1, 0, 2).view(np.uint16))
            return u

        # seg0: per-k tiles [k, p, hl, u]
        b0 = blob(*SEGS[0])
        m = {"xk": np.ascontiguousarray(b0.transpose(1, 0, 2, 3)),
             "w1r": w1u, "cblob": cblk,
             "eo": np.ascontiguousarray(
                 eo[:, sl, :].reshape(E * T, H).astype(np.float16))}
        for s in range(1, NSEG):
            m[f"xs{s}"] = np.ascontiguousarray(blob(*SEGS[s]))
        in_maps.append(m)
    return in_maps


def kernel(hidden_states, expert_outputs, W1, b1, W2, b2, k=2):
    from concourse.bass_utils import run_bass_kernel_spmd

    in_maps = make_in_maps(hidden_states, expert_outputs, W1, b1, W2, b2)
    nc = _get_nc()
    res = run_bass_kernel_spmd(nc, in_maps, core_ids=list(range(N_CORES)))
    full = np.concatenate([res.results[c]["out"].astype(np.float32)
                           for c in range(N_CORES)], axis=0)
    return full.reshape(B, S, H)
